# revision 18
# baseline (speedup 1.0000x reference)
"""Trainium2 Bass kernel for nn_PairwiseAttentionTerminal.

Reference computation (L=1024, B=8, F=256, H=8, C=32):
    x = layernorm(features)                       # (L, B, F)
    q,k,v = x@Wq+bq, x@Wk+bk, x@Wv+bv             # (L, B, H, C)
    bias  = x@Wb+bb                               # (L, B, H) per-key bias
    gate  = sigmoid(x@Wg+bg)                      # (L, B, H, C)
    S     = einsum('qbhc,kbhc->qbkh', q, k)/sqrt(C) + bias[None]
    attn  = softmax_k(S) @ v                      # (L, B, H, C)
    out   = (attn*gate) @ Wo + bo                 # (L, B, F)

Sharding: batch B=8 -> one batch element per NeuronCore (8 cores), weights
replicated, no collectives.  Host shards/gathers around one SPMD NEFF.

Per-core design (v2):
  - The ACT engine is the hard floor: softmax needs exp of L*L*H = 8.4M
    logits = 64 x [128,1024] Exp ops (~66us).  Everything else is arranged
    to overlap under that stream.
  - Bias fold: exp(s + b_k) = exp(s) * exp(b_k); exp(b_k) is absorbed into
    the ones-augmented V columns (per-key scale), so the softmax Exp ops
    have no bias operand and nothing blocks them but the S matmul.
  - LN statistics on DVE (tensor_tensor_reduce for E[x^2]); ACT only does
    the tiny rstd ln/exp pairs.  Ramp work (PSUM->SBUF copies, V-augment
    assembly) is spread across ACT/DVE/Pool so the serial ramp before the
    exp stream is as short as possible.
  - gate = sigmoid(y) = e^y/(1+e^y): one ACT Exp pass per F-chunk plus two
    DVE passes (+1, divide) -- no ln/exp round trips.
  - Softmax normalization via DVE `divide`: denominators (ones-row of the
    augmented-V matmul) are copied out on Pool, broadcast partition-wise
    with gpsimd partition_broadcast (heads 0-6, off critical path) or a
    K=1 PE broadcast (head 7, tail), then one tensor_tensor divide per
    head group.  No reciprocal, no DRAM round-trip broadcast DMAs.
  - Zero-bias specialization: the effective projection biases are checked
    host-side; all-zero biases (the common case here) skip the bias-add
    passes entirely (projection PSUM->SBUF moves become plain copies).
  - S^T per (head, k-tile) with K=32 contraction at tile_position=(ph,0);
    1/sqrt(C) folded into Wq host-side.  All big matmuls float32r.
  - Only ACT table set used is natural_log_exp_and_others (one load).
"""

import numpy as np
from contextlib import ExitStack

L, B, F, H, C = 1024, 8, 256, 8, 32
HC = H * C
EPS = 1e-5
N_CORES = 8
P = 128
NLT = L // P  # 8 L-tiles
NFC = F // P  # 2 F-chunks

_COMPILED = {}


def _build(zb):
    """zb: dict of zero-flags for effective biases (qk, v, b, g, o)."""
    import concourse.bacc as bacc
    import concourse.mybir as mybir
    import concourse.tile as tile

    f32 = mybir.dt.float32
    f32r = mybir.dt.float32r
    AF = mybir.ActivationFunctionType
    ALU = mybir.AluOpType

    nc = bacc.Bacc("TRN2", target_bir_lowering=False)

    # ---- DRAM I/O (per-core) ----
    feat_e = nc.dram_tensor("feat", [L, F], f32, kind="ExternalInput")
    wq_e = nc.dram_tensor("wq", [P, NFC, HC], f32r, kind="ExternalInput")
    wk_e = nc.dram_tensor("wk", [P, NFC, HC], f32r, kind="ExternalInput")
    wv_e = nc.dram_tensor("wv", [P, NFC, HC], f32r, kind="ExternalInput")
    wg_e = nc.dram_tensor("wg", [P, NFC, HC], f32r, kind="ExternalInput")
    wb_e = nc.dram_tensor("wb", [P, NFC, H], f32r, kind="ExternalInput")
    wo_e = nc.dram_tensor("wo", [P, NFC, F], f32r, kind="ExternalInput")
    bq_e = nc.dram_tensor("bq_t", [P, NFC], f32, kind="ExternalInput")
    bk_e = nc.dram_tensor("bk_t", [P, NFC], f32, kind="ExternalInput")
    bg_e = nc.dram_tensor("bg_t", [P, NFC], f32, kind="ExternalInput")
    bv_e = nc.dram_tensor("bv_b", [P, F], f32, kind="ExternalInput")
    bb_e = nc.dram_tensor("bb_b", [P, H], f32, kind="ExternalInput")
    bo_e = nc.dram_tensor("bo_b", [P, F], f32, kind="ExternalInput")
    id_e = nc.dram_tensor("ident", [P, P], f32, kind="ExternalInput")
    onesr_e = nc.dram_tensor("onesr", [1, P], f32r, kind="ExternalInput")
    out_e = nc.dram_tensor("out", [L, F], f32, kind="ExternalOutput")

    with tile.TileContext(nc) as tc, ExitStack() as ctx:
        const = ctx.enter_context(tc.tile_pool(name="const", bufs=1))
        main = ctx.enter_context(tc.tile_pool(name="main", bufs=1))
        work = ctx.enter_context(tc.tile_pool(name="work", bufs=4))
        epool = ctx.enter_context(tc.tile_pool(name="epool", bufs=3))
        opool = ctx.enter_context(tc.tile_pool(name="opool", bufs=6))

        # ---- features + constants; ident early on the Pool queue ----
        ftp = ctx.enter_context(tc.tile_pool(name="ftp", bufs=1))
        ft = [ftp.tile([P, F], f32, name=f"ft{i}") for i in range(NLT)]
        ident = const.tile([P, P], f32, name="id_s")
        nc.gpsimd.dma_start(ident[:], id_e.ap())
        for i in range(NLT):
            nc.sync.dma_start(ft[i][:], feat_e.ap()[i * P:(i + 1) * P, :])

        def load(name, ext, shape, dt_=f32, eng=None):
            t = const.tile(shape, dt_, name=name)
            (eng or nc.sync).dma_start(t[:], ext.ap())
            return t

        wv = load("wv_s", wv_e, [P, NFC, HC], f32r)
        wb = load("wb_s", wb_e, [P, NFC, H], f32r)
        wq = load("wq_s", wq_e, [P, NFC, HC], f32r)
        wk = load("wk_s", wk_e, [P, NFC, HC], f32r)
        wg = load("wg_s", wg_e, [P, NFC, HC], f32r)
        wo = load("wo_s", wo_e, [P, NFC, F], f32r)
        onesr = load("onesr", onesr_e, [1, P], f32r, eng=nc.gpsimd)
        bq = bk = bg = bvb = bbb = bob = None
        if not zb["qk"]:
            bq = load("bq_s", bq_e, [P, NFC], eng=nc.gpsimd)
            bk = load("bk_s", bk_e, [P, NFC], eng=nc.gpsimd)
        if not zb["g"]:
            bg = load("bg_s", bg_e, [P, NFC], eng=nc.gpsimd)
        if not zb["v"]:
            bvb = load("bv_s", bv_e, [P, F], eng=nc.gpsimd)
        if not zb["b"]:
            bbb = load("bb_s", bb_e, [P, H], eng=nc.gpsimd)
        if not zb["o"]:
            bob = load("bo_s", bo_e, [P, F], eng=nc.gpsimd)
        epst = const.tile([P, 1], f32, name="epst")
        nc.vector.memset(epst[:], EPS)

        # ---- persistent big tiles ----
        xT = [main.tile([P, L], f32r, name=f"xT{j}") for j in range(NFC)]
        qT = [main.tile([P, L], f32r, name=f"qT{j}") for j in range(NFC)]
        kT = [main.tile([P, L], f32r, name=f"kT{j}") for j in range(NFC)]
        gT = [main.tile([P, L], f32, name=f"gT{j}") for j in range(NFC)]
        agu = [main.tile([P, L], f32, name=f"agu{j}") for j in range(NFC)]
        agT = [main.tile([P, L], f32r, name=f"agT{j}") for j in range(NFC)]
        vaug = [main.tile([P, H, C + 1], f32r, name=f"vaug{i}") for i in range(NLT)]
        dh = [main.tile([1, L], f32r, name=f"dh{h}") for h in range(H)]
        dBs = [main.tile([P, L], f32r, name=f"dBs{b_}") for b_ in range(2)]
        # e-storage for heads 0/1, k-tiles 0..3: [P, kk, q]; written in
        # m-half column blocks by paired exps (2 k-tiles per [128,1024] op)
        ea = [main.tile([P, 4, L], f32r, name=f"ea{h}") for h in range(2)]

        psA_cm = tc.tile_pool(name="psA", bufs=2, space="PSUM")
        psA = psA_cm.__enter__()
        psAB_cm = tc.tile_pool(name="psAB", bufs=2, space="PSUM")
        psAB = psAB_cm.__enter__()
        psal_cm = tc.tile_pool(name="psal", bufs=1, space="PSUM")
        psal = psal_cm.__enter__()
        apH = [None] * H

        def qk_proj(m):
            ms = slice(512 * m, 512 * (m + 1))
            # k first (S matmuls consume kT tiles first), copies spread over
            # ACT/DVE/Pool; bias add only if nonzero.
            plan = [(wk, bk, kT, 0, "act"), (wq, bq, qT, 0, "pool"),
                    (wk, bk, kT, 1, "pool"), (wq, bq, qT, 1, "dve")]
            for (w, bvec, dst, j, eng) in plan:
                ps = psAB.tile([P, 512], f32, tag="ab", name=f"p{m}{j}")
                nc.tensor.matmul(ps[:], w[:, 0, j * P:(j + 1) * P],
                                 xT[0][:, ms], start=True, stop=False)
                nc.tensor.matmul(ps[:], w[:, 1, j * P:(j + 1) * P],
                                 xT[1][:, ms], start=False, stop=True)
                if zb["qk"]:
                    if eng == "act":
                        nc.scalar.activation(dst[j][:, ms], ps[:], AF.Copy)
                    elif eng == "pool":
                        nc.gpsimd.tensor_copy(dst[j][:, ms], ps[:])
                    else:
                        nc.vector.tensor_copy(dst[j][:, ms], ps[:])
                else:
                    nc.vector.tensor_scalar(dst[j][:, ms], ps[:],
                                            bvec[:, j:j + 1], None, op0=ALU.add)

        # ============ Stage A: LN + transpose + v/bias proj ============
        for i in range(NLT):
            ls = slice(i * P, (i + 1) * P)
            st = work.tile([P, 8], f32, tag="st")
            sq = work.tile([P, F], f32, tag="sq")
            nc.vector.tensor_reduce(st[:, 0:1], ft[i][:],
                                    axis=mybir.AxisListType.X, op=ALU.add)
            nc.vector.tensor_tensor_reduce(sq[:], ft[i][:], ft[i][:], 1.0, 0.0,
                                           ALU.mult, ALU.add, st[:, 2:3])
            nc.vector.tensor_scalar_mul(st[:, 1:2], st[:, 0:1], 1.0 / F)
            nc.vector.tensor_tensor(st[:, 5:6], st[:, 1:2], st[:, 1:2],
                                    op=ALU.mult)
            nc.vector.tensor_scalar(st[:, 6:7], st[:, 2:3], 1.0 / F,
                                    st[:, 5:6], op0=ALU.mult, op1=ALU.subtract)
            # rstd = exp(-0.5 * ln(var + eps)) (single ACT table set)
            nc.scalar.activation(st[:, 3:4], st[:, 6:7], AF.Ln, bias=epst[:])
            nc.scalar.activation(st[:, 4:5], st[:, 3:4], AF.Exp, scale=-0.5)
            xn = work.tile([P, F], f32, tag="xn")
            nc.vector.tensor_scalar(xn[:], ft[i][:], st[:, 1:2], st[:, 4:5],
                                    op0=ALU.subtract, op1=ALU.mult)
            tp0 = psAB.tile([P, P], f32, tag="ab", name=f"tp{i}_0")
            nc.tensor.transpose(tp0[:], xn[:, 0:P], ident[:])
            nc.gpsimd.tensor_copy(xT[0][:, ls], tp0[:])
            tp1 = psAB.tile([P, P], f32, tag="ab", name=f"tp{i}_1")
            nc.tensor.transpose(tp1[:], xn[:, P:F], ident[:])
            nc.gpsimd.tensor_copy(xT[1][:, ls], tp1[:])
            # v projection + per-key bias projection for this L-slice
            psv = psAB.tile([P, F], f32, tag="ab", name=f"pv{i}")
            nc.tensor.matmul(psv[:], xT[0][:, ls], wv[:, 0, :],
                             start=True, stop=False)
            nc.tensor.matmul(psv[:], xT[1][:, ls], wv[:, 1, :],
                             start=False, stop=True)
            psb = psAB.tile([P, H], f32, tag="ab", name=f"pb{i}")
            nc.tensor.matmul(psb[:], xT[0][:, ls], wb[:, 0, :],
                             start=True, stop=False)
            nc.tensor.matmul(psb[:], xT[1][:, ls], wb[:, 1, :],
                             start=False, stop=True)
            # expb = exp(bias_k); fold into augmented V (cols 0..C-1 scaled,
            # col C holds expb itself -> ones-row sum gives the softmax denom)
            eb = work.tile([P, H], f32, tag="eb")
            if zb["b"]:
                nc.scalar.activation(eb[:], psb[:], AF.Exp)
            else:
                bt = work.tile([P, H], f32, tag="bt")
                nc.vector.tensor_tensor(bt[:], psb[:], bbb[:], op=ALU.add)
                nc.scalar.activation(eb[:], bt[:], AF.Exp)
            if zb["v"]:
                nc.gpsimd.tensor_tensor(
                    vaug[i][:, :, 0:C],
                    psv[:].rearrange("p (h c) -> p h c", h=H),
                    eb[:].to_broadcast([P, H, C]), op=ALU.mult)
            else:
                nc.vector.tensor_tensor(
                    vaug[i][:, :, 0:C],
                    psv[:].rearrange("p (h c) -> p h c", h=H),
                    bvb[:].rearrange("p (h c) -> p h c", h=H), op=ALU.add)
                nc.vector.tensor_tensor(
                    vaug[i][:, :, 0:C], vaug[i][:, :, 0:C],
                    eb[:].to_broadcast([P, H, C]), op=ALU.mult)
            nc.gpsimd.tensor_copy(vaug[i][:, :, C], eb[:])
            if i == 3:
                qk_proj(0)
                # alpha: heads 0/1, k-tiles 0-3, q-half m0 -- start the exp
                # stream while L-tiles 4-7 are still in layernorm.  Two
                # k-tiles share one [128,1024] Exp op (full-size, no per-op
                # overhead penalty) via the [P, kk, q] ea layout.
                for h in range(2):
                    ph = 32 * h
                    hp = slice(ph, ph + 32)
                    apH[h] = psA.tile([33, L], f32, tag="a", name=f"ap{h}")
                    for kp in range(2):
                        spr = psal.tile([P, L], f32, tag="al", name=f"al{h}{kp}")
                        for u in range(2):
                            kk = 2 * kp + u
                            ks = slice(kk * P, (kk + 1) * P)
                            nc.tensor.matmul(spr[:, 512 * u:512 * (u + 1)],
                                             kT[0][hp, ks], qT[0][hp, 0:512],
                                             start=True, stop=True,
                                             tile_position=(ph, 0))
                        nc.scalar.activation(
                            ea[h][:, 2 * kp:2 * kp + 2, 0:512],
                            spr[:].rearrange("p (a b) -> p a b", a=2), AF.Exp)
                        for u in range(2):
                            kk = 2 * kp + u
                            nc.tensor.matmul(apH[h][:, 0:512],
                                             vaug[kk][:, h, :],
                                             ea[h][:, kk, 0:512],
                                             start=(kk == 0), stop=False)
        qk_proj(1)

        psal_cm.__exit__(None, None, None)
        psAB_cm.__exit__(None, None, None)
        psS_cm = tc.tile_pool(name="psS", bufs=2, space="PSUM")
        psS = psS_cm.__enter__()

        def gate_emit():
            # gate = sigmoid(x@Wg + bg) = e^y/(1+e^y)
            for j in range(NFC):
                pg = psS.tile([P, L], f32, tag="s", name=f"pg{j}")
                for m in range(2):
                    ms = slice(512 * m, 512 * (m + 1))
                    nc.tensor.matmul(pg[:, ms], wg[:, 0, j * P:(j + 1) * P],
                                     xT[0][:, ms], start=True, stop=False)
                    nc.tensor.matmul(pg[:, ms], wg[:, 1, j * P:(j + 1) * P],
                                     xT[1][:, ms], start=False, stop=True)
                eg = work.tile([P, L], f32, tag="eg", name=f"eg{j}")
                if zb["g"]:
                    nc.scalar.activation(eg[:], pg[:], AF.Exp)
                else:
                    nc.scalar.activation(eg[:], pg[:], AF.Exp,
                                         bias=bg[:, j:j + 1])
                tg = work.tile([P, L], f32, tag="tg", name=f"tg{j}")
                nc.vector.tensor_scalar(tg[:], eg[:], 1.0, None, op0=ALU.add)
                nc.vector.tensor_tensor(gT[j][:], eg[:], tg[:], op=ALU.divide)

        # beta halves: heads 0/1, k-tiles 0-3, q-half m1
        for h in range(2):
            ph = 32 * h
            hp = slice(ph, ph + 32)
            for kp in range(2):
                spr = psS.tile([P, L], f32, tag="s", name=f"bt{h}{kp}")
                for u in range(2):
                    kk = 2 * kp + u
                    ks = slice(kk * P, (kk + 1) * P)
                    nc.tensor.matmul(spr[:, 512 * u:512 * (u + 1)],
                                     kT[0][hp, ks], qT[0][hp, 512:1024],
                                     start=True, stop=True,
                                     tile_position=(ph, 0))
                nc.scalar.activation(
                    ea[h][:, 2 * kp:2 * kp + 2, 512:1024],
                    spr[:].rearrange("p (a b) -> p a b", a=2), AF.Exp)
                for u in range(2):
                    kk = 2 * kp + u
                    nc.tensor.matmul(apH[h][:, 512:1024], vaug[kk][:, h, :],
                                     ea[h][:, kk, 512:1024],
                                     start=(kk == 0), stop=False)
            if h == 0:
                gate_emit()

        # ============ Attention head loop ============
        for h in range(H):
            jh, ph = h // 4, 32 * (h % 4)
            hp = slice(ph, ph + 32)
            if h < 2:
                ap = apH[h]
                kk_list = range(4, NLT)
            else:
                ap = psA.tile([33, L], f32, tag="a", name=f"ap{h}")
                kk_list = range(NLT)
            for kk in kk_list:
                ks = slice(kk * P, (kk + 1) * P)
                sp = psS.tile([P, L], f32, tag="s", name=f"sp{h}{kk}")
                for m in range(2):
                    ms = slice(512 * m, 512 * (m + 1))
                    nc.tensor.matmul(sp[:, ms], kT[jh][hp, ks],
                                     qT[jh][hp, ms], start=True, stop=True,
                                     tile_position=(ph, 0))
                eT = epool.tile([P, L], f32r, tag="e", name=f"e{h}{kk}")
                nc.scalar.activation(eT[:], sp[:], AF.Exp)
                for m in range(2):
                    ms = slice(512 * m, 512 * (m + 1))
                    nc.tensor.matmul(ap[:, ms], vaug[kk][:, h, :], eT[:, ms],
                                     start=(kk == 0 and h >= 2),
                                     stop=(kk == NLT - 1))
            # drain: fold gate; stash denominator row + partition-broadcast it
            if h < 7:
                nc.vector.tensor_tensor(agu[jh][hp, :], ap[0:32, :],
                                        gT[jh][hp, :], op=ALU.mult)
                nc.gpsimd.tensor_copy(dh[h][:], ap[32:33, :])
                nc.gpsimd.partition_broadcast(dBs[jh][hp, :], dh[h][:])
            if h == 3:
                nc.vector.tensor_tensor(agT[0][:], agu[0][:], dBs[0][:],
                                        op=ALU.divide)
            if h == 6:
                nc.vector.tensor_tensor(agT[1][0:96, :], agu[1][0:96, :],
                                        dBs[1][0:96, :], op=ALU.divide)
            if h == 7:
                # tail head: everything in L-halves on disjoint tiles so the
                # drain, K=1 PE denominator broadcast, divide, and the first
                # output-projection slices pipeline with no false WAR deps
                dB7 = [psS.tile([P, L // 2], f32, tag="s", name=f"dB7{m}")
                       for m in range(2)]
                for m in range(2):
                    ms = slice(512 * m, 512 * (m + 1))
                    nc.gpsimd.tensor_copy(dh[7][0:1, ms], ap[32:33, ms])
                    nc.vector.tensor_tensor(agu[1][96:128, ms],
                                            ap[0:32, ms],
                                            gT[1][96:128, ms], op=ALU.mult)
                    nc.tensor.matmul(dB7[m][0:32, :], onesr[0:1, 0:32],
                                     dh[7][0:1, ms], start=True, stop=True,
                                     tile_position=(0, 0))
                    nc.vector.tensor_tensor(agT[1][96:128, ms],
                                            agu[1][96:128, ms],
                                            dB7[m][0:32, :], op=ALU.divide)

        # ============ Output projection ============
        out_q = [nc.sync, nc.scalar, nc.gpsimd, nc.sync,
                 nc.scalar, nc.gpsimd, nc.scalar, nc.sync]
        o_eng = ["pool", "act", "dve", "pool", "act", "dve", "pool", "act"]
        for i in range(NLT):
            ls = slice(i * P, (i + 1) * P)
            ps = psS.tile([P, F], f32, tag="s", name=f"po{i}")
            nc.tensor.matmul(ps[:], agT[0][:, ls], wo[:, 0, :],
                             start=True, stop=False)
            nc.tensor.matmul(ps[:], agT[1][:, ls], wo[:, 1, :],
                             start=False, stop=True)
            o = opool.tile([P, F], f32, tag="o", name=f"o{i}")
            if zb["o"]:
                if o_eng[i] == "pool":
                    nc.gpsimd.tensor_copy(o[:], ps[:])
                elif o_eng[i] == "act":
                    nc.scalar.activation(o[:], ps[:], AF.Copy)
                else:
                    nc.vector.tensor_copy(o[:], ps[:])
            else:
                nc.vector.tensor_tensor(o[:], ps[:], bob[:], op=ALU.add)
            out_q[i].dma_start(out_e.ap()[ls, :], o[:])

        psS_cm.__exit__(None, None, None)
        psA_cm.__exit__(None, None, None)

    # The act-table chooser greedily picks the FIRST set containing each
    # function, thrashing between exp_and_others and natural_log.  Restrict
    # Exp/Ln/Square/Copy to the one combined set so a single table load
    # serves the whole kernel (ids/order preserved).
    import concourse.bacc as bacc_mod
    orig_gat = bacc_mod.get_activation_tables

    def gat_combined(arch):
        t = orig_gat(arch)
        out = {}
        drop = {AF.Exp, AF.Ln, AF.Square, AF.Copy}
        for name, funcs in t.items():
            if name == "natural_log_exp_and_others":
                out[name] = funcs
            else:
                out[name] = funcs - drop
        return out

    bacc_mod.get_activation_tables = gat_combined
    try:
        nc.compile()
    finally:
        bacc_mod.get_activation_tables = orig_gat
    return nc


def _prep_inputs(features, ln_g, ln_b, Wq, bq, Wk, bk, Wv, bv, Wb, bb,
                 Wg, bg, Wo, bo):
    f32 = np.float32
    sq = f32(1.0 / np.sqrt(C))
    g_ = np.asarray(ln_g, f32)[:, None]
    b_ = np.asarray(ln_b, f32)

    def wsplit(W, n):
        return np.ascontiguousarray(
            np.asarray(W, f32).reshape(NFC, P, n).transpose(1, 0, 2))

    def bsplit(b):
        return np.ascontiguousarray(np.asarray(b, f32).reshape(NFC, P).T)

    Wq_ = np.asarray(Wq, f32) * g_ * sq
    bq_ = (b_ @ (np.asarray(Wq, f32) * sq) + np.asarray(bq, f32) * sq)
    Wk_ = np.asarray(Wk, f32) * g_
    bk_ = b_ @ np.asarray(Wk, f32) + np.asarray(bk, f32)
    Wv_ = np.asarray(Wv, f32) * g_
    bv_ = b_ @ np.asarray(Wv, f32) + np.asarray(bv, f32)
    Wg_ = np.asarray(Wg, f32) * g_
    bg_ = b_ @ np.asarray(Wg, f32) + np.asarray(bg, f32)
    Wb_ = np.asarray(Wb, f32) * g_
    bb_ = b_ @ np.asarray(Wb, f32) + np.asarray(bb, f32)
    bo_ = np.asarray(bo, f32)

    zb = {
        "qk": bool(np.all(bq_ == 0) and np.all(bk_ == 0)),
        "v": bool(np.all(bv_ == 0)),
        "b": bool(np.all(bb_ == 0)),
        "g": bool(np.all(bg_ == 0)),
        "o": bool(np.all(bo_ == 0)),
    }

    common = {
        "wq": wsplit(Wq_, HC),
        "wk": wsplit(Wk_, HC),
        "wv": wsplit(Wv_, HC),
        "wg": wsplit(Wg_, HC),
        "wb": wsplit(Wb_, H),
        "wo": wsplit(Wo, F),
        "bq_t": bsplit(bq_),
        "bk_t": bsplit(bk_),
        "bg_t": bsplit(bg_),
        "bv_b": np.ascontiguousarray(np.tile(bv_, (P, 1))),
        "bb_b": np.ascontiguousarray(np.tile(np.asarray(bb_, f32), (P, 1))),
        "bo_b": np.ascontiguousarray(np.tile(bo_, (P, 1))),
        "ident": np.eye(P, dtype=f32),
        "onesr": np.ones((1, P), f32),
    }
    feats = np.asarray(features, f32)
    in_maps = []
    for b_i in range(N_CORES):
        m = dict(common)
        m["feat"] = np.ascontiguousarray(feats[:, b_i, :])
        in_maps.append(m)
    return in_maps, zb


def kernel(**inputs):
    from concourse.bass_utils import run_bass_kernel_spmd

    in_maps, zb = _prep_inputs(**inputs)
    key = tuple(sorted(zb.items()))
    if key not in _COMPILED:
        _COMPILED[key] = _build(zb)
        _COMPILED["nc"] = _COMPILED[key]  # convenience handle for test.py
    nc = _COMPILED[key]
    res = run_bass_kernel_spmd(nc, in_maps, list(range(N_CORES)))
    out = np.stack([res.results[b_]["out"] for b_ in range(N_CORES)], axis=1)
    return np.ascontiguousarray(out.astype(np.float32))


if __name__ == "__main__":
    rng = np.random.default_rng(0)
    ins = {
        "features": rng.standard_normal((L, B, F), dtype=np.float32),
        "ln_g": np.ones(F, np.float32), "ln_b": np.zeros(F, np.float32),
        "Wq": rng.standard_normal((F, HC), dtype=np.float32) * 0.02,
        "bq": np.zeros(HC, np.float32),
        "Wk": rng.standard_normal((F, HC), dtype=np.float32) * 0.02,
        "bk": np.zeros(HC, np.float32),
        "Wv": rng.standard_normal((F, HC), dtype=np.float32) * 0.02,
        "bv": np.zeros(HC, np.float32),
        "Wb": rng.standard_normal((F, H), dtype=np.float32) * 0.02,
        "bb": np.zeros(H, np.float32),
        "Wg": rng.standard_normal((F, HC), dtype=np.float32) * 0.02,
        "bg": np.zeros(HC, np.float32),
        "Wo": rng.standard_normal((HC, F), dtype=np.float32) * 0.02,
        "bo": np.zeros(F, np.float32),
    }
    print(kernel(**ins).shape)


# revision 26
# speedup vs baseline: 1.0502x; 1.0502x over previous
"""Trainium2 Bass kernel for nn_PairwiseAttentionTerminal.

Reference computation (L=1024, B=8, F=256, H=8, C=32):
    x = layernorm(features)                       # (L, B, F)
    q,k,v = x@Wq+bq, x@Wk+bk, x@Wv+bv             # (L, B, H, C)
    bias  = x@Wb+bb                               # (L, B, H) per-key bias
    gate  = sigmoid(x@Wg+bg)                      # (L, B, H, C)
    S     = einsum('qbhc,kbhc->qbkh', q, k)/sqrt(C) + bias[None]
    attn  = softmax_k(S) @ v                      # (L, B, H, C)
    out   = (attn*gate) @ Wo + bo                 # (L, B, F)

Sharding: batch B=8 -> one batch element per NeuronCore (8 cores), weights
replicated, no collectives.  Host shards/gathers around one SPMD NEFF.

Per-core design (v2):
  - The ACT engine is the hard floor: softmax needs exp of L*L*H = 8.4M
    logits = 64 x [128,1024] Exp ops (~66us).  Everything else is arranged
    to overlap under that stream.
  - Bias fold: exp(s + b_k) = exp(s) * exp(b_k); exp(b_k) is absorbed into
    the ones-augmented V columns (per-key scale), so the softmax Exp ops
    have no bias operand and nothing blocks them but the S matmul.
  - LN statistics on DVE (tensor_tensor_reduce for E[x^2]); ACT only does
    the tiny rstd ln/exp pairs.  Ramp work (PSUM->SBUF copies, V-augment
    assembly) is spread across ACT/DVE/Pool so the serial ramp before the
    exp stream is as short as possible.
  - gate = sigmoid(y) = e^y/(1+e^y): one ACT Exp pass per F-chunk plus two
    DVE passes (+1, divide) -- no ln/exp round trips.
  - Softmax normalization via DVE `divide`: denominators (ones-row of the
    augmented-V matmul) are copied out on Pool, broadcast partition-wise
    with gpsimd partition_broadcast (heads 0-6, off critical path) or a
    K=1 PE broadcast (head 7, tail), then one tensor_tensor divide per
    head group.  No reciprocal, no DRAM round-trip broadcast DMAs.
  - Zero-bias specialization: the effective projection biases are checked
    host-side; all-zero biases (the common case here) skip the bias-add
    passes entirely (projection PSUM->SBUF moves become plain copies).
  - S^T per (head, k-tile) with K=32 contraction at tile_position=(ph,0);
    1/sqrt(C) folded into Wq host-side.  All big matmuls float32r.
  - Only ACT table set used is natural_log_exp_and_others (one load).
"""

import numpy as np
from contextlib import ExitStack

L, B, F, H, C = 1024, 8, 256, 8, 32
HC = H * C
EPS = 1e-5
N_CORES = 8
P = 128
NLT = L // P  # 8 L-tiles
NFC = F // P  # 2 F-chunks

_COMPILED = {}


def _build(zb):
    """zb: dict of zero-flags for effective biases (qk, v, b, g, o)."""
    import concourse.bacc as bacc
    import concourse.mybir as mybir
    import concourse.tile as tile

    f32 = mybir.dt.float32
    f32r = mybir.dt.float32r
    AF = mybir.ActivationFunctionType
    ALU = mybir.AluOpType

    nc = bacc.Bacc("TRN2", target_bir_lowering=False)

    # ---- DRAM I/O (per-core) ----
    feat_e = nc.dram_tensor("feat", [L, F], f32, kind="ExternalInput")
    wq_e = nc.dram_tensor("wq", [P, NFC, HC], f32r, kind="ExternalInput")
    wk_e = nc.dram_tensor("wk", [P, NFC, HC], f32r, kind="ExternalInput")
    wv_e = nc.dram_tensor("wv", [P, NFC, HC], f32r, kind="ExternalInput")
    wg_e = nc.dram_tensor("wg", [P, NFC, HC], f32r, kind="ExternalInput")
    wb_e = nc.dram_tensor("wb", [P, NFC, H], f32r, kind="ExternalInput")
    wo_e = nc.dram_tensor("wo", [P, NFC, F], f32r, kind="ExternalInput")
    bq_e = nc.dram_tensor("bq_t", [P, NFC], f32, kind="ExternalInput")
    bk_e = nc.dram_tensor("bk_t", [P, NFC], f32, kind="ExternalInput")
    bg_e = nc.dram_tensor("bg_t", [P, NFC], f32, kind="ExternalInput")
    bv_e = nc.dram_tensor("bv_b", [P, F], f32, kind="ExternalInput")
    bb_e = nc.dram_tensor("bb_b", [P, H], f32, kind="ExternalInput")
    bo_e = nc.dram_tensor("bo_b", [P, F], f32, kind="ExternalInput")
    id_e = nc.dram_tensor("ident", [P, P], f32, kind="ExternalInput")
    onesr_e = nc.dram_tensor("onesr", [1, P], f32r, kind="ExternalInput")
    out_e = nc.dram_tensor("out", [L, F], f32, kind="ExternalOutput")

    with tile.TileContext(nc) as tc, ExitStack() as ctx:
        const = ctx.enter_context(tc.tile_pool(name="const", bufs=1))
        main = ctx.enter_context(tc.tile_pool(name="main", bufs=1))
        work = ctx.enter_context(tc.tile_pool(name="work", bufs=4))
        epool = ctx.enter_context(tc.tile_pool(name="epool", bufs=3))
        opool = ctx.enter_context(tc.tile_pool(name="opool", bufs=6))

        # ---- features + constants; ident early on the Pool queue ----
        ftp = ctx.enter_context(tc.tile_pool(name="ftp", bufs=1))
        ft = [ftp.tile([P, F], f32, name=f"ft{i}") for i in range(NLT)]
        ident = const.tile([P, P], f32, name="id_s")
        nc.gpsimd.dma_start(ident[:], id_e.ap())
        for i in range(NLT):
            nc.sync.dma_start(ft[i][:], feat_e.ap()[i * P:(i + 1) * P, :])

        def load(name, ext, shape, dt_=f32, eng=None):
            t = const.tile(shape, dt_, name=name)
            (eng or nc.sync).dma_start(t[:], ext.ap())
            return t

        wv = load("wv_s", wv_e, [P, NFC, HC], f32r)
        wb = load("wb_s", wb_e, [P, NFC, H], f32r)
        wq = load("wq_s", wq_e, [P, NFC, HC], f32r)
        wk = load("wk_s", wk_e, [P, NFC, HC], f32r)
        wg = load("wg_s", wg_e, [P, NFC, HC], f32r)
        wo = load("wo_s", wo_e, [P, NFC, F], f32r)
        onesr = load("onesr", onesr_e, [1, P], f32r, eng=nc.gpsimd)
        bq = bk = bg = bvb = bbb = bob = None
        if not zb["qk"]:
            bq = load("bq_s", bq_e, [P, NFC], eng=nc.gpsimd)
            bk = load("bk_s", bk_e, [P, NFC], eng=nc.gpsimd)
        if not zb["g"]:
            bg = load("bg_s", bg_e, [P, NFC], eng=nc.gpsimd)
        if not zb["v"]:
            bvb = load("bv_s", bv_e, [P, F], eng=nc.gpsimd)
        if not zb["b"]:
            bbb = load("bb_s", bb_e, [P, H], eng=nc.gpsimd)
        if not zb["o"]:
            bob = load("bo_s", bo_e, [P, F], eng=nc.gpsimd)
        epst = const.tile([P, 1], f32, name="epst")
        nc.vector.memset(epst[:], EPS)

        # ---- persistent big tiles ----
        xT = [main.tile([P, L], f32r, name=f"xT{j}") for j in range(NFC)]
        qT = [main.tile([P, L], f32r, name=f"qT{j}") for j in range(NFC)]
        kT = [main.tile([P, L], f32r, name=f"kT{j}") for j in range(NFC)]
        gT = [main.tile([P, L], f32, name=f"gT{j}") for j in range(NFC)]
        agu = [main.tile([P, L], f32, name=f"agu{j}") for j in range(NFC)]
        agT = [main.tile([P, L], f32r, name=f"agT{j}") for j in range(NFC)]
        vaug = [main.tile([P, H, C + 1], f32r, name=f"vaug{i}") for i in range(NLT)]
        dh = [main.tile([1, L], f32r, name=f"dh{h}") for h in range(H)]
        dBs = [main.tile([P, L], f32r, name=f"dBs{b_}") for b_ in range(2)]
        # e-storage for heads 0/1, k-tiles 0..3: [P, kk, q]; written in
        # m-half column blocks by paired exps (2 k-tiles per [128,1024] op)
        ea = [main.tile([P, 4, L], f32r, name=f"ea{h}") for h in range(2)]

        psAB_cm = tc.tile_pool(name="psAB", bufs=4, space="PSUM")
        psAB = psAB_cm.__enter__()
        psal_cm = tc.tile_pool(name="psal", bufs=2, space="PSUM")
        psal = psal_cm.__enter__()
        apH = [None] * H

        def qk_proj(m):
            ms = slice(512 * m, 512 * (m + 1))
            # k first (S matmuls consume kT tiles first), copies spread over
            # ACT/DVE/Pool; bias add only if nonzero.
            if m == 0:
                plan = [(wk, bk, kT, 0, "dve"), (wq, bq, qT, 0, "pool"),
                        (wk, bk, kT, 1, "pool"), (wq, bq, qT, 1, "dve")]
            else:
                plan = [(wq, bq, qT, 0, "pool"), (wk, bk, kT, 0, "dve"),
                        (wk, bk, kT, 1, "pool"), (wq, bq, qT, 1, "dve")]
            for (w, bvec, dst, j, eng) in plan:
                ps = psAB.tile([P, 512], f32, tag="ab", name=f"p{m}{j}")
                nc.tensor.matmul(ps[:], w[:, 0, j * P:(j + 1) * P],
                                 xT[0][:, ms], start=True, stop=False)
                nc.tensor.matmul(ps[:], w[:, 1, j * P:(j + 1) * P],
                                 xT[1][:, ms], start=False, stop=True)
                if zb["qk"]:
                    if eng == "act":
                        nc.scalar.activation(dst[j][:, ms], ps[:], AF.Copy)
                    elif eng == "pool":
                        nc.gpsimd.tensor_copy(dst[j][:, ms], ps[:])
                    else:
                        nc.vector.tensor_copy(dst[j][:, ms], ps[:])
                else:
                    nc.vector.tensor_scalar(dst[j][:, ms], ps[:],
                                            bvec[:, j:j + 1], None, op0=ALU.add)

        # ============ Stage A: LN + transpose + v/bias proj ============
        # Stats split DVE/ACT by tile parity; xn on Pool -- three engines
        # chew the layernorm pipeline concurrently so the alpha exp stream
        # can start as early as possible.
        for i in range(NLT):
            ls = slice(i * P, (i + 1) * P)
            st = work.tile([P, 8], f32, tag="st")
            sq = work.tile([P, F], f32, tag="sq")
            nc.vector.tensor_reduce(st[:, 0:1], ft[i][:],
                                    axis=mybir.AxisListType.X, op=ALU.add)
            if i % 2 == 0:
                nc.vector.tensor_tensor_reduce(sq[:], ft[i][:], ft[i][:],
                                               1.0, 0.0, ALU.mult, ALU.add,
                                               st[:, 2:3])
            else:
                nc.scalar.activation(sq[:], ft[i][:], AF.Square,
                                     accum_out=st[:, 2:3])
            nc.vector.tensor_scalar_mul(st[:, 1:2], st[:, 0:1], 1.0 / F)
            nc.vector.tensor_tensor(st[:, 5:6], st[:, 1:2], st[:, 1:2],
                                    op=ALU.mult)
            nc.vector.tensor_scalar(st[:, 6:7], st[:, 2:3], 1.0 / F,
                                    st[:, 5:6], op0=ALU.mult, op1=ALU.subtract)
            # rstd = exp(-0.5 * ln(var + eps)) (single ACT table set)
            nc.scalar.activation(st[:, 3:4], st[:, 6:7], AF.Ln, bias=epst[:])
            nc.scalar.activation(st[:, 4:5], st[:, 3:4], AF.Exp, scale=-0.5)
            xn = work.tile([P, F], f32, tag="xn")
            nc.gpsimd.tensor_scalar(xn[:], ft[i][:], st[:, 1:2], st[:, 4:5],
                                    op0=ALU.subtract, op1=ALU.mult)
            tp0 = psAB.tile([P, P], f32, tag="ab", name=f"tp{i}_0")
            nc.tensor.transpose(tp0[:], xn[:, 0:P], ident[:])
            nc.gpsimd.tensor_copy(xT[0][:, ls], tp0[:])
            tp1 = psAB.tile([P, P], f32, tag="ab", name=f"tp{i}_1")
            nc.tensor.transpose(tp1[:], xn[:, P:F], ident[:])
            nc.gpsimd.tensor_copy(xT[1][:, ls], tp1[:])
            # v projection + per-key bias projection for this L-slice
            psv = psAB.tile([P, F], f32, tag="ab", name=f"pv{i}")
            nc.tensor.matmul(psv[:], xT[0][:, ls], wv[:, 0, :],
                             start=True, stop=False)
            nc.tensor.matmul(psv[:], xT[1][:, ls], wv[:, 1, :],
                             start=False, stop=True)
            psb = psAB.tile([P, H], f32, tag="ab", name=f"pb{i}")
            nc.tensor.matmul(psb[:], xT[0][:, ls], wb[:, 0, :],
                             start=True, stop=False)
            nc.tensor.matmul(psb[:], xT[1][:, ls], wb[:, 1, :],
                             start=False, stop=True)
            # expb = exp(bias_k); fold into augmented V (cols 0..C-1 scaled,
            # col C holds expb itself -> ones-row sum gives the softmax denom)
            eb = work.tile([P, H], f32, tag="eb")
            if zb["b"]:
                nc.scalar.activation(eb[:], psb[:], AF.Exp)
            else:
                bt = work.tile([P, H], f32, tag="bt")
                nc.vector.tensor_tensor(bt[:], psb[:], bbb[:], op=ALU.add)
                nc.scalar.activation(eb[:], bt[:], AF.Exp)
            if zb["v"]:
                nc.gpsimd.tensor_tensor(
                    vaug[i][:, :, 0:C],
                    psv[:].rearrange("p (h c) -> p h c", h=H),
                    eb[:].to_broadcast([P, H, C]), op=ALU.mult)
            else:
                nc.vector.tensor_tensor(
                    vaug[i][:, :, 0:C],
                    psv[:].rearrange("p (h c) -> p h c", h=H),
                    bvb[:].rearrange("p (h c) -> p h c", h=H), op=ALU.add)
                nc.vector.tensor_tensor(
                    vaug[i][:, :, 0:C], vaug[i][:, :, 0:C],
                    eb[:].to_broadcast([P, H, C]), op=ALU.mult)
            nc.gpsimd.tensor_copy(vaug[i][:, :, C], eb[:])
            if i == 3:
                qk_proj(0)
                # alpha: heads 0/1, k-tiles 0-3, q-half m0 -- start the exp
                # stream while L-tiles 4-7 are still in layernorm.  Two
                # k-tiles share one [128,1024] Exp op (full-size, no per-op
                # overhead penalty) via the [P, kk, q] ea layout.  The attn
                # matmuls catch up in beta once psA opens.
                for h in range(2):
                    ph = 32 * h
                    hp = slice(ph, ph + 32)
                    for kp in range(2):
                        spr = psal.tile([P, L], f32, tag="al", name=f"al{h}{kp}")
                        for u in range(2):
                            kk = 2 * kp + u
                            ks = slice(kk * P, (kk + 1) * P)
                            nc.tensor.matmul(spr[:, 512 * u:512 * (u + 1)],
                                             kT[0][hp, ks], qT[0][hp, 0:512],
                                             start=True, stop=True,
                                             tile_position=(ph, 0))
                        nc.scalar.activation(
                            ea[h][:, 2 * kp:2 * kp + 2, 0:512],
                            spr[:].rearrange("p (a b) -> p a b", a=2), AF.Exp)
        qk_proj(1)

        psal_cm.__exit__(None, None, None)
        psAB_cm.__exit__(None, None, None)
        psA_cm = tc.tile_pool(name="psA", bufs=2, space="PSUM")
        psA = psA_cm.__enter__()
        psS_cm = tc.tile_pool(name="psS", bufs=2, space="PSUM")
        psS = psS_cm.__enter__()

        def gate_emit():
            # gate = sigmoid(x@Wg + bg) = e^y/(1+e^y)
            for j in range(NFC):
                pg = psS.tile([P, L], f32, tag="s", name=f"pg{j}")
                for m in range(2):
                    ms = slice(512 * m, 512 * (m + 1))
                    nc.tensor.matmul(pg[:, ms], wg[:, 0, j * P:(j + 1) * P],
                                     xT[0][:, ms], start=True, stop=False)
                    nc.tensor.matmul(pg[:, ms], wg[:, 1, j * P:(j + 1) * P],
                                     xT[1][:, ms], start=False, stop=True)
                eg = work.tile([P, L], f32, tag="eg", name=f"eg{j}")
                if zb["g"]:
                    nc.scalar.activation(eg[:], pg[:], AF.Exp)
                else:
                    nc.scalar.activation(eg[:], pg[:], AF.Exp,
                                         bias=bg[:, j:j + 1])
                tg = work.tile([P, L], f32, tag="tg", name=f"tg{j}")
                nc.vector.tensor_scalar(tg[:], eg[:], 1.0, None, op0=ALU.add)
                nc.vector.tensor_tensor(gT[j][:], eg[:], tg[:], op=ALU.divide)

        # beta: heads 0/1 -- attn catch-up for the alpha half, then the
        # remaining m1 half of k-tiles 0-3
        for h in range(2):
            ph = 32 * h
            hp = slice(ph, ph + 32)
            apH[h] = psA.tile([33, L], f32, tag="a", name=f"ap{h}")
            for kk in range(4):
                nc.tensor.matmul(apH[h][:, 0:512], vaug[kk][:, h, :],
                                 ea[h][:, kk, 0:512],
                                 start=(kk == 0), stop=False)
            for kp in range(2):
                spr = psS.tile([P, L], f32, tag="s", name=f"bt{h}{kp}")
                for u in range(2):
                    kk = 2 * kp + u
                    ks = slice(kk * P, (kk + 1) * P)
                    nc.tensor.matmul(spr[:, 512 * u:512 * (u + 1)],
                                     kT[0][hp, ks], qT[0][hp, 512:1024],
                                     start=True, stop=True,
                                     tile_position=(ph, 0))
                nc.scalar.activation(
                    ea[h][:, 2 * kp:2 * kp + 2, 512:1024],
                    spr[:].rearrange("p (a b) -> p a b", a=2), AF.Exp)
                for u in range(2):
                    kk = 2 * kp + u
                    nc.tensor.matmul(apH[h][:, 512:1024], vaug[kk][:, h, :],
                                     ea[h][:, kk, 512:1024],
                                     start=(kk == 0), stop=False)
            if h == 0:
                gate_emit()

        # ============ Attention head loop ============
        for h in range(H):
            jh, ph = h // 4, 32 * (h % 4)
            hp = slice(ph, ph + 32)
            if h < 2:
                ap = apH[h]
                kk_list = range(4, NLT)
            else:
                ap = psA.tile([33, L], f32, tag="a", name=f"ap{h}")
                kk_list = range(NLT)
            for kk in kk_list:
                ks = slice(kk * P, (kk + 1) * P)
                sp = psS.tile([P, L], f32, tag="s", name=f"sp{h}{kk}")
                for m in range(2):
                    ms = slice(512 * m, 512 * (m + 1))
                    nc.tensor.matmul(sp[:, ms], kT[jh][hp, ks],
                                     qT[jh][hp, ms], start=True, stop=True,
                                     tile_position=(ph, 0))
                eT = epool.tile([P, L], f32r, tag="e", name=f"e{h}{kk}")
                nc.scalar.activation(eT[:], sp[:], AF.Exp)
                for m in range(2):
                    ms = slice(512 * m, 512 * (m + 1))
                    nc.tensor.matmul(ap[:, ms], vaug[kk][:, h, :], eT[:, ms],
                                     start=(kk == 0 and h >= 2),
                                     stop=(kk == NLT - 1))
            # drain: fold gate; stash denominator row + partition-broadcast it
            if h < 7:
                nc.vector.tensor_tensor(agu[jh][hp, :], ap[0:32, :],
                                        gT[jh][hp, :], op=ALU.mult)
                nc.gpsimd.tensor_copy(dh[h][:], ap[32:33, :])
                nc.gpsimd.partition_broadcast(dBs[jh][hp, :], dh[h][:])
            if h == 3:
                nc.gpsimd.tensor_tensor(agT[0][:], agu[0][:], dBs[0][:],
                                        op=ALU.divide)
            if h == 6:
                nc.gpsimd.tensor_tensor(agT[1][0:96, :], agu[1][0:96, :],
                                        dBs[1][0:96, :], op=ALU.divide)
            if h == 7:
                # tail head: everything in L-halves on disjoint tiles so the
                # drain, K=1 PE denominator broadcast, divide, and the first
                # output-projection slices pipeline with no false WAR deps
                dB7 = [psS.tile([P, L // 2], f32, tag="s", name=f"dB7{m}")
                       for m in range(2)]
                mul_e = [nc.gpsimd, nc.vector]
                div_e = [nc.vector, nc.gpsimd]
                for m in range(2):
                    ms = slice(512 * m, 512 * (m + 1))
                    (nc.gpsimd if m == 0 else nc.vector).tensor_copy(
                        dh[7][0:1, ms], ap[32:33, ms])
                    mul_e[m].tensor_tensor(agu[1][96:128, ms],
                                           ap[0:32, ms],
                                           gT[1][96:128, ms], op=ALU.mult)
                    nc.tensor.matmul(dB7[m][0:32, :], onesr[0:1, 0:32],
                                     dh[7][0:1, ms], start=True, stop=True,
                                     tile_position=(0, 0))
                    div_e[m].tensor_tensor(agT[1][96:128, ms],
                                           agu[1][96:128, ms],
                                           dB7[m][0:32, :], op=ALU.divide)

        # ============ Output projection ============
        out_q = [nc.sync, nc.scalar, nc.gpsimd, nc.sync,
                 nc.scalar, nc.gpsimd, nc.scalar, nc.sync]
        o_eng = ["pool", "act", "dve", "pool", "act", "dve", "pool", "act"]
        for i in range(NLT):
            ls = slice(i * P, (i + 1) * P)
            ps = psS.tile([P, F], f32, tag="s", name=f"po{i}")
            nc.tensor.matmul(ps[:], agT[0][:, ls], wo[:, 0, :],
                             start=True, stop=False)
            nc.tensor.matmul(ps[:], agT[1][:, ls], wo[:, 1, :],
                             start=False, stop=True)
            o = opool.tile([P, F], f32, tag="o", name=f"o{i}")
            if zb["o"]:
                if o_eng[i] == "pool":
                    nc.gpsimd.tensor_copy(o[:], ps[:])
                elif o_eng[i] == "act":
                    nc.scalar.activation(o[:], ps[:], AF.Copy)
                else:
                    nc.vector.tensor_copy(o[:], ps[:])
            else:
                nc.vector.tensor_tensor(o[:], ps[:], bob[:], op=ALU.add)
            out_q[i].dma_start(out_e.ap()[ls, :], o[:])

        psS_cm.__exit__(None, None, None)
        psA_cm.__exit__(None, None, None)

    # The act-table chooser greedily picks the FIRST set containing each
    # function, thrashing between exp_and_others and natural_log.  Restrict
    # Exp/Ln/Square/Copy to the one combined set so a single table load
    # serves the whole kernel (ids/order preserved).
    import concourse.bacc as bacc_mod
    orig_gat = bacc_mod.get_activation_tables

    def gat_combined(arch):
        t = orig_gat(arch)
        out = {}
        drop = {AF.Exp, AF.Ln, AF.Square, AF.Copy}
        for name, funcs in t.items():
            if name == "natural_log_exp_and_others":
                out[name] = funcs
            else:
                out[name] = funcs - drop
        return out

    bacc_mod.get_activation_tables = gat_combined
    try:
        nc.compile()
    finally:
        bacc_mod.get_activation_tables = orig_gat
    return nc


def _prep_inputs(features, ln_g, ln_b, Wq, bq, Wk, bk, Wv, bv, Wb, bb,
                 Wg, bg, Wo, bo):
    f32 = np.float32
    sq = f32(1.0 / np.sqrt(C))
    g_ = np.asarray(ln_g, f32)[:, None]
    b_ = np.asarray(ln_b, f32)

    def wsplit(W, n):
        return np.ascontiguousarray(
            np.asarray(W, f32).reshape(NFC, P, n).transpose(1, 0, 2))

    def bsplit(b):
        return np.ascontiguousarray(np.asarray(b, f32).reshape(NFC, P).T)

    Wq_ = np.asarray(Wq, f32) * g_ * sq
    bq_ = (b_ @ (np.asarray(Wq, f32) * sq) + np.asarray(bq, f32) * sq)
    Wk_ = np.asarray(Wk, f32) * g_
    bk_ = b_ @ np.asarray(Wk, f32) + np.asarray(bk, f32)
    Wv_ = np.asarray(Wv, f32) * g_
    bv_ = b_ @ np.asarray(Wv, f32) + np.asarray(bv, f32)
    Wg_ = np.asarray(Wg, f32) * g_
    bg_ = b_ @ np.asarray(Wg, f32) + np.asarray(bg, f32)
    Wb_ = np.asarray(Wb, f32) * g_
    bb_ = b_ @ np.asarray(Wb, f32) + np.asarray(bb, f32)
    bo_ = np.asarray(bo, f32)

    zb = {
        "qk": bool(np.all(bq_ == 0) and np.all(bk_ == 0)),
        "v": bool(np.all(bv_ == 0)),
        "b": bool(np.all(bb_ == 0)),
        "g": bool(np.all(bg_ == 0)),
        "o": bool(np.all(bo_ == 0)),
    }

    common = {
        "wq": wsplit(Wq_, HC),
        "wk": wsplit(Wk_, HC),
        "wv": wsplit(Wv_, HC),
        "wg": wsplit(Wg_, HC),
        "wb": wsplit(Wb_, H),
        "wo": wsplit(Wo, F),
        "bq_t": bsplit(bq_),
        "bk_t": bsplit(bk_),
        "bg_t": bsplit(bg_),
        "bv_b": np.ascontiguousarray(np.tile(bv_, (P, 1))),
        "bb_b": np.ascontiguousarray(np.tile(np.asarray(bb_, f32), (P, 1))),
        "bo_b": np.ascontiguousarray(np.tile(bo_, (P, 1))),
        "ident": np.eye(P, dtype=f32),
        "onesr": np.ones((1, P), f32),
    }
    feats = np.asarray(features, f32)
    in_maps = []
    for b_i in range(N_CORES):
        m = dict(common)
        m["feat"] = np.ascontiguousarray(feats[:, b_i, :])
        in_maps.append(m)
    return in_maps, zb


def kernel(**inputs):
    from concourse.bass_utils import run_bass_kernel_spmd

    in_maps, zb = _prep_inputs(**inputs)
    key = tuple(sorted(zb.items()))
    if key not in _COMPILED:
        _COMPILED[key] = _build(zb)
        _COMPILED["nc"] = _COMPILED[key]  # convenience handle for test.py
    nc = _COMPILED[key]
    res = run_bass_kernel_spmd(nc, in_maps, list(range(N_CORES)))
    out = np.stack([res.results[b_]["out"] for b_ in range(N_CORES)], axis=1)
    return np.ascontiguousarray(out.astype(np.float32))


if __name__ == "__main__":
    rng = np.random.default_rng(0)
    ins = {
        "features": rng.standard_normal((L, B, F), dtype=np.float32),
        "ln_g": np.ones(F, np.float32), "ln_b": np.zeros(F, np.float32),
        "Wq": rng.standard_normal((F, HC), dtype=np.float32) * 0.02,
        "bq": np.zeros(HC, np.float32),
        "Wk": rng.standard_normal((F, HC), dtype=np.float32) * 0.02,
        "bk": np.zeros(HC, np.float32),
        "Wv": rng.standard_normal((F, HC), dtype=np.float32) * 0.02,
        "bv": np.zeros(HC, np.float32),
        "Wb": rng.standard_normal((F, H), dtype=np.float32) * 0.02,
        "bb": np.zeros(H, np.float32),
        "Wg": rng.standard_normal((F, HC), dtype=np.float32) * 0.02,
        "bg": np.zeros(HC, np.float32),
        "Wo": rng.standard_normal((HC, F), dtype=np.float32) * 0.02,
        "bo": np.zeros(F, np.float32),
    }
    print(kernel(**ins).shape)
